# revision 11
# baseline (speedup 1.0000x reference)
"""Trainium2 Bass kernel for the binarized conv1d + maxpool + PReLU + BatchNorm block.

Reference computation (per full input):
  x: [256, 64, 4096] f32, W: [128, 64, 7], alpha: [1], gamma/beta: [128]
  xp = pad(x, 3 each side, value=-1)
  y  = conv1d(sign(xp), sign(W), VALID)          -> [256, 128, 4096]
  y  = maxpool(y, k=2, s=2)                      -> [256, 128, 2048]
  y  = prelu(y, alpha)
  y  = batchnorm_train(y, gamma, beta)  (stats over batch and length)

Strategy: data-parallel over batch, 32 batches per core on 8 NeuronCores.
Conv is done as 4 accumulating matmuls per 512-column tile: contraction
(i=64 channels) x (k tap pairs) packed into 128 SBUF partitions by keeping
two copies of the signed signal (shifted by 0 and 1 columns) on partitions
0-63 / 64-127. BN statistics are all-reduced across cores ([128,2] floats).
"""

import sys

sys.path.insert(0, "/opt/trn_rl_repo")

import numpy as np
import ml_dtypes

from contextlib import ExitStack

import concourse.bass as bass
import concourse.tile as tile
from concourse import bacc, mybir
from concourse.bass_utils import run_bass_kernel_spmd

N_CORES = 8
B_FULL = 256
B_LOC = B_FULL // N_CORES  # 32
C_IN = 64
C_OUT = 128
L_IN = 4096
L_OUT = L_IN // 2  # 2048
KSIZE = 7
PADDING = 3
PAD_VAL = -1.0
BN_EPS = 1e-5
LP = L_IN + 2 * PADDING  # 4102 padded signal length
A_W = 4104  # A tile width (LP rounded up to a multiple of 8)
N_TILE = 512  # conv cols per matmul (one PSUM bank)
HALF = L_IN // 2  # conv cols per PSUM tile (4 banks)

F32 = mybir.dt.float32
F16 = mybir.dt.float16
BF16 = mybir.dt.bfloat16


def _build_program(alpha_val: float, n_batches: int = B_LOC, skip: frozenset = frozenset()):
    nc = bacc.Bacc("TRN2", target_bir_lowering=False, debug=False, num_devices=N_CORES)

    x_in = nc.declare_dram_parameter("x", [B_LOC, 128, L_IN // 2], F32, isOutput=False)
    w_in = nc.declare_dram_parameter("w", [128, 4 * 128], BF16, isOutput=False)
    gamma_in = nc.declare_dram_parameter("gamma", [128, 1], F32, isOutput=False)
    beta_in = nc.declare_dram_parameter("beta", [128, 1], F32, isOutput=False)
    out_d = nc.declare_dram_parameter("out", [B_LOC, C_OUT, L_OUT], F32, isOutput=True)

    cc_in = nc.dram_tensor("cc_in", [128, 2], F32)
    cc_out = nc.dram_tensor("cc_out", [128, 2], F32, addr_space="Shared")

    x_ap = x_in.ap()
    w_ap = w_in.ap()
    out_ap = out_d.ap()

    with tile.TileContext(nc) as tc, ExitStack() as ctx:
        consts = ctx.enter_context(tc.tile_pool(name="consts", bufs=1))
        statsp = ctx.enter_context(tc.tile_pool(name="stats", bufs=1))
        ybig = ctx.enter_context(tc.tile_pool(name="ybig", bufs=1))
        xin = ctx.enter_context(tc.tile_pool(name="xin", bufs=2))
        sgn = ctx.enter_context(tc.tile_pool(name="sgn", bufs=2))
        sqp = ctx.enter_context(tc.tile_pool(name="sqp", bufs=2))
        atile = ctx.enter_context(tc.tile_pool(name="atile", bufs=2))
        ztile = ctx.enter_context(tc.tile_pool(name="ztile", bufs=2))
        outp = ctx.enter_context(tc.tile_pool(name="outp", bufs=2))
        psum = ctx.enter_context(tc.tile_pool(name="psum", bufs=2, space="PSUM"))

        w_sb = consts.tile([128, 4 * 128], BF16)
        nc.sync.dma_start(out=w_sb[:], in_=w_ap[:])
        gamma_sb = consts.tile([128, 1], F32)
        nc.sync.dma_start(out=gamma_sb[:], in_=gamma_in.ap()[:])
        beta_sb = consts.tile([128, 1], F32)
        nc.sync.dma_start(out=beta_sb[:], in_=beta_in.ap()[:])

        sums = statsp.tile([128, B_LOC], F32)
        sumsqs = statsp.tile([128, B_LOC], F32)
        Y = ybig.tile([128, B_LOC * L_OUT], F16)

        # ---------------- Phase 1: conv + pool + prelu + per-batch stats ----
        for b in range(n_batches):
            X = xin.tile([128, L_IN // 2], F32)
            nc.sync.dma_start(out=X[:], in_=x_ap[b])

            S = sgn.tile([128, L_IN // 2], BF16)
            nc.scalar.activation(S[:], X[:], mybir.ActivationFunctionType.Sign)

            # A holds the signed, padded signal: partitions 0-63 at shift 0,
            # partitions 64-127 at shift +1 (for packing two conv taps into
            # one 128-partition contraction).
            A = atile.tile([128, A_W], BF16)
            nc.vector.memset(A[:, 0:PADDING], PAD_VAL)
            nc.vector.memset(A[:, LP - 4 : A_W], PAD_VAL)
            h = L_IN // 2  # 2048
            nc.sync.dma_start(out=A[0:64, 3 : 3 + h], in_=S[0:128:2, :])
            nc.sync.dma_start(out=A[0:64, 3 + h : 3 + 2 * h], in_=S[1:128:2, :])
            nc.sync.dma_start(out=A[64:128, 2 : 2 + h], in_=S[0:128:2, :])
            nc.sync.dma_start(out=A[64:128, 2 + h : 2 + 2 * h], in_=S[1:128:2, :])

            Z = ztile.tile([128, L_OUT], BF16)
            for half in range(2):
                P = psum.tile([128, HALF], F32)
                for j in range(HALF // N_TILE):
                    c = HALF * half + N_TILE * j
                    o = N_TILE * j
                    nc.tensor.matmul(
                        P[:, o : o + N_TILE],
                        w_sb[:, 0:128],
                        A[:, c : c + N_TILE],
                        start=True,
                        stop=False,
                    )
                    nc.tensor.matmul(
                        P[:, o : o + N_TILE],
                        w_sb[:, 128:256],
                        A[:, c + 2 : c + 2 + N_TILE],
                        start=False,
                        stop=False,
                    )
                    nc.tensor.matmul(
                        P[:, o : o + N_TILE],
                        w_sb[:, 256:384],
                        A[:, c + 4 : c + 4 + N_TILE],
                        start=False,
                        stop="k64" in skip,
                    )
                    if "k64" not in skip:
                        nc.tensor.matmul(
                            P[:, o : o + N_TILE],
                            w_sb[0:64, 384:512],
                            A[0:64, c + 6 : c + 6 + N_TILE],
                            start=False,
                            stop=True,
                        )
                # max-pool k=2 s=2 straight out of PSUM (single-input reduce —
                # TensorTensor may read only one operand from PSUM)
                nc.vector.tensor_reduce(
                    out=Z[:, half * (HALF // 2) : (half + 1) * (HALF // 2)],
                    in_=P[:].rearrange("p (l two) -> p l two", two=2),
                    axis=mybir.AxisListType.X,
                    op=mybir.AluOpType.max,
                )

            Yb = Y[:, b * L_OUT : (b + 1) * L_OUT]
            if "prelu" in skip:
                nc.scalar.activation(
                    Yb,
                    Z[:],
                    mybir.ActivationFunctionType.Identity,
                    accum_out=sums[:, b : b + 1],
                )
            else:
                # PReLU with fused per-partition sum accumulation
                nc.scalar.activation(
                    Yb,
                    Z[:],
                    mybir.ActivationFunctionType.Prelu,
                    alpha=alpha_val,
                    accum_out=sums[:, b : b + 1],
                )
            if "ttr" in skip:
                nc.vector.memset(sumsqs[:, b : b + 1], float(L_OUT))
            else:
                # sum of squares: (y * 1.0) * y with fused per-partition sum
                SQ = sqp.tile([128, L_OUT], BF16)
                nc.vector.scalar_tensor_tensor(
                    out=SQ[:],
                    in0=Yb,
                    scalar=1.0,
                    in1=Yb,
                    op0=mybir.AluOpType.mult,
                    op1=mybir.AluOpType.mult,
                    accum_out=sumsqs[:, b : b + 1],
                )

        # ---------------- Phase 2: global BN stats + scale/shift ------------
        if n_batches < B_LOC:
            nc.vector.memset(sums[:, n_batches:B_LOC], 0.0)
            nc.vector.memset(sumsqs[:, n_batches:B_LOC], 0.0)
        sm = statsp.tile([128, 2], F32)
        nc.vector.tensor_reduce(
            sm[:, 0:1], sums[:], axis=mybir.AxisListType.X, op=mybir.AluOpType.add
        )
        nc.vector.tensor_reduce(
            sm[:, 1:2], sumsqs[:], axis=mybir.AxisListType.X, op=mybir.AluOpType.add
        )
        sg = statsp.tile([128, 2], F32)
        if "cc" in skip:
            nc.vector.tensor_scalar_mul(sg[:], sm[:], float(N_CORES))
        else:
            nc.sync.dma_start(out=cc_in[:], in_=sm[:])
            nc.gpsimd.collective_compute(
                "AllReduce",
                mybir.AluOpType.add,
                replica_groups=[list(range(N_CORES))],
                ins=[cc_in[:]],
                outs=[cc_out[:]],
            )
            nc.sync.dma_start(out=sg[:], in_=cc_out[:])

        inv_n = 1.0 / float(N_CORES * n_batches * L_OUT)
        mean = statsp.tile([128, 1], F32)
        nc.vector.tensor_scalar_mul(mean[:], sg[:, 0:1], inv_n)
        ve = statsp.tile([128, 1], F32)
        # ve = E[y^2] - mean^2 + eps   via (sg1*inv_n + eps) - mean^2
        e2 = statsp.tile([128, 1], F32)
        nc.vector.tensor_scalar(
            e2[:], sg[:, 1:2], inv_n, BN_EPS, mybir.AluOpType.mult, mybir.AluOpType.add
        )
        msq = statsp.tile([128, 1], F32)
        nc.vector.tensor_mul(msq[:], mean[:], mean[:])
        nc.vector.tensor_sub(ve[:], e2[:], msq[:])
        # rstd = 1/sqrt(ve), with one Newton step to clean up ACT sqrt error
        sq = statsp.tile([128, 1], F32)
        nc.scalar.activation(sq[:], ve[:], mybir.ActivationFunctionType.Sqrt)
        r0 = statsp.tile([128, 1], F32)
        nc.vector.reciprocal(r0[:], sq[:])
        rr = statsp.tile([128, 1], F32)
        nc.vector.tensor_mul(rr[:], r0[:], r0[:])
        nc.vector.tensor_mul(rr[:], rr[:], ve[:])
        nc.vector.tensor_scalar(
            rr[:], rr[:], -0.5, 1.5, mybir.AluOpType.mult, mybir.AluOpType.add
        )
        rstd = statsp.tile([128, 1], F32)
        nc.vector.tensor_mul(rstd[:], r0[:], rr[:])
        s_vec = statsp.tile([128, 1], F32)
        nc.vector.tensor_mul(s_vec[:], rstd[:], gamma_sb[:])
        t_vec = statsp.tile([128, 1], F32)
        nc.vector.tensor_mul(t_vec[:], mean[:], s_vec[:])
        nc.vector.tensor_sub(t_vec[:], beta_sb[:], t_vec[:])

        # ---------------- Phase 3: apply BN affine and store -----------------
        for b in range(n_batches):
            O = outp.tile([128, L_OUT], F32)
            nc.scalar.activation(
                O[:],
                Y[:, b * L_OUT : (b + 1) * L_OUT],
                mybir.ActivationFunctionType.Identity,
                bias=t_vec[:],
                scale=s_vec[:],
            )
            nc.sync.dma_start(out=out_ap[b], in_=O[:])

    nc.compile()
    return nc


def _prep_weights(W: np.ndarray) -> np.ndarray:
    sW = np.sign(W).astype(np.float32)  # [128, 64, 7]
    w_host = np.zeros((128, 4 * 128), dtype=np.float32)
    for t in range(3):
        w_host[0:64, 128 * t : 128 * (t + 1)] = sW[:, :, 2 * t].T
        w_host[64:128, 128 * t : 128 * (t + 1)] = sW[:, :, 2 * t + 1].T
    w_host[0:64, 384:512] = sW[:, :, 6].T
    return w_host.astype(ml_dtypes.bfloat16)


def kernel(x, W, alpha, gamma, beta):
    x = np.asarray(x, dtype=np.float32)
    W = np.asarray(W, dtype=np.float32)
    alpha_val = float(np.asarray(alpha).reshape(-1)[0])
    gamma = np.asarray(gamma, dtype=np.float32).reshape(128, 1)
    beta = np.asarray(beta, dtype=np.float32).reshape(128, 1)

    nc = _build_program(alpha_val)
    w_host = _prep_weights(W)

    in_maps = []
    for c in range(N_CORES):
        xs = np.ascontiguousarray(x[c * B_LOC : (c + 1) * B_LOC]).reshape(
            B_LOC, 128, L_IN // 2
        )
        in_maps.append({"x": xs, "w": w_host, "gamma": gamma, "beta": beta})

    res = run_bass_kernel_spmd(nc, in_maps, list(range(N_CORES)))
    out = np.concatenate([res.results[c]["out"] for c in range(N_CORES)], axis=0)
    return out.astype(np.float32)


if __name__ == "__main__":
    rng = np.random.default_rng(0)
    x = rng.standard_normal((B_FULL, C_IN, L_IN), dtype=np.float32)
    W = rng.standard_normal((C_OUT, C_IN, KSIZE), dtype=np.float32)
    alpha = np.full((1,), 0.25, np.float32)
    gamma = np.ones((C_OUT,), np.float32)
    beta = np.zeros((C_OUT,), np.float32)
    out = kernel(x=x, W=W, alpha=alpha, gamma=gamma, beta=beta)
    print(out.shape, out.dtype, float(out.mean()), float(out.std()))


# revision 12
# speedup vs baseline: 1.2148x; 1.2148x over previous
"""Trainium2 Bass kernel for the binarized conv1d + maxpool + PReLU + BatchNorm block.

Reference computation (per full input):
  x: [256, 64, 4096] f32, W: [128, 64, 7], alpha: [1], gamma/beta: [128]
  xp = pad(x, 3 each side, value=-1)
  y  = conv1d(sign(xp), sign(W), VALID)          -> [256, 128, 4096]
  y  = maxpool(y, k=2, s=2)                      -> [256, 128, 2048]
  y  = prelu(y, alpha)
  y  = batchnorm_train(y, gamma, beta)  (stats over batch and length)

Strategy: data-parallel over batch, 32 batches per core on 8 NeuronCores.
Conv is done as 4 accumulating matmuls per 512-column tile: contraction
(i=64 channels) x (k tap pairs) packed into 128 SBUF partitions by keeping
two copies of the signed signal (shifted by 0 and 1 columns) on partitions
0-63 / 64-127. BN statistics are all-reduced across cores ([128,2] floats).
"""

import sys

sys.path.insert(0, "/opt/trn_rl_repo")

import numpy as np
import ml_dtypes

from contextlib import ExitStack

import concourse.bass as bass
import concourse.tile as tile
from concourse import bacc, mybir
from concourse.bass_utils import run_bass_kernel_spmd

N_CORES = 8
B_FULL = 256
B_LOC = B_FULL // N_CORES  # 32
C_IN = 64
C_OUT = 128
L_IN = 4096
L_OUT = L_IN // 2  # 2048
KSIZE = 7
PADDING = 3
PAD_VAL = -1.0
BN_EPS = 1e-5
LP = L_IN + 2 * PADDING  # 4102 padded signal length
A_W = 4104  # A tile width (LP rounded up to a multiple of 8)
N_TILE = 512  # conv cols per matmul (one PSUM bank)
HALF = L_IN // 2  # conv cols per PSUM tile (4 banks)

F32 = mybir.dt.float32
F16 = mybir.dt.float16
BF16 = mybir.dt.bfloat16
FP8 = mybir.dt.float8e4


def _build_program(alpha_val: float, n_batches: int = B_LOC, skip: frozenset = frozenset()):
    nc = bacc.Bacc("TRN2", target_bir_lowering=False, debug=False, num_devices=N_CORES)

    x_in = nc.declare_dram_parameter("x", [B_LOC, 128, L_IN // 2], F32, isOutput=False)
    w_in = nc.declare_dram_parameter("w", [128, 4 * 128], FP8, isOutput=False)
    gamma_in = nc.declare_dram_parameter("gamma", [128, 1], F32, isOutput=False)
    beta_in = nc.declare_dram_parameter("beta", [128, 1], F32, isOutput=False)
    out_d = nc.declare_dram_parameter("out", [B_LOC, C_OUT, L_OUT], F32, isOutput=True)

    cc_in = nc.dram_tensor("cc_in", [128, 2], F32)
    cc_out = nc.dram_tensor("cc_out", [128, 2], F32, addr_space="Shared")

    x_ap = x_in.ap()
    w_ap = w_in.ap()
    out_ap = out_d.ap()

    with tile.TileContext(nc) as tc, ExitStack() as ctx:
        consts = ctx.enter_context(tc.tile_pool(name="consts", bufs=1))
        statsp = ctx.enter_context(tc.tile_pool(name="stats", bufs=1))
        ybig = ctx.enter_context(tc.tile_pool(name="ybig", bufs=1))
        xin = ctx.enter_context(tc.tile_pool(name="xin", bufs=3))
        sgn = ctx.enter_context(tc.tile_pool(name="sgn", bufs=2))
        sqp = ctx.enter_context(tc.tile_pool(name="sqp", bufs=2))
        atile = ctx.enter_context(tc.tile_pool(name="atile", bufs=2))
        ztile = ctx.enter_context(tc.tile_pool(name="ztile", bufs=2))
        outp = ctx.enter_context(tc.tile_pool(name="outp", bufs=2))
        psum = ctx.enter_context(tc.tile_pool(name="psum", bufs=2, space="PSUM"))

        w_sb = consts.tile([128, 4 * 128], FP8)
        nc.sync.dma_start(out=w_sb[:], in_=w_ap[:])
        gamma_sb = consts.tile([128, 1], F32)
        nc.sync.dma_start(out=gamma_sb[:], in_=gamma_in.ap()[:])
        beta_sb = consts.tile([128, 1], F32)
        nc.sync.dma_start(out=beta_sb[:], in_=beta_in.ap()[:])

        sums = statsp.tile([128, B_LOC], F32)
        sumsqs = statsp.tile([128, B_LOC], F32)
        Y = ybig.tile([128, B_LOC * L_OUT], F16)

        # ---------------- Phase 1: conv + pool + prelu + per-batch stats ----
        for b in range(n_batches):
            X = xin.tile([128, L_IN // 2], F32)
            nc.sync.dma_start(out=X[:], in_=x_ap[b])

            S = sgn.tile([128, L_IN // 2], FP8)
            nc.scalar.activation(S[:], X[:], mybir.ActivationFunctionType.Sign)

            # A holds the signed, padded signal: partitions 0-63 at shift 0,
            # partitions 64-127 at shift +1 (for packing two conv taps into
            # one 128-partition contraction).
            A = atile.tile([128, A_W], FP8)
            nc.vector.memset(A[:, 0:PADDING], PAD_VAL)
            nc.vector.memset(A[:, LP - 4 : A_W], PAD_VAL)
            h = L_IN // 2  # 2048
            nc.sync.dma_start(out=A[0:64, 3 : 3 + h], in_=S[0:128:2, :])
            nc.sync.dma_start(out=A[0:64, 3 + h : 3 + 2 * h], in_=S[1:128:2, :])
            nc.sync.dma_start(out=A[64:128, 2 : 2 + h], in_=S[0:128:2, :])
            nc.sync.dma_start(out=A[64:128, 2 + h : 2 + 2 * h], in_=S[1:128:2, :])

            Z = ztile.tile([128, L_OUT], BF16)
            for half in range(2):
                P = psum.tile([128, HALF], F32)
                for j in range(HALF // N_TILE):
                    c = HALF * half + N_TILE * j
                    o = N_TILE * j
                    nc.tensor.matmul(
                        P[:, o : o + N_TILE],
                        w_sb[:, 0:128],
                        A[:, c : c + N_TILE],
                        start=True,
                        stop=False,
                    )
                    nc.tensor.matmul(
                        P[:, o : o + N_TILE],
                        w_sb[:, 128:256],
                        A[:, c + 2 : c + 2 + N_TILE],
                        start=False,
                        stop=False,
                    )
                    nc.tensor.matmul(
                        P[:, o : o + N_TILE],
                        w_sb[:, 256:384],
                        A[:, c + 4 : c + 4 + N_TILE],
                        start=False,
                        stop="k64" in skip,
                    )
                    if "k64" not in skip:
                        nc.tensor.matmul(
                            P[:, o : o + N_TILE],
                            w_sb[:, 384:512],
                            A[:, c + 6 : c + 6 + N_TILE],
                            start=False,
                            stop=True,
                        )
                # max-pool k=2 s=2 straight out of PSUM (single-input reduce —
                # TensorTensor may read only one operand from PSUM)
                nc.vector.tensor_reduce(
                    out=Z[:, half * (HALF // 2) : (half + 1) * (HALF // 2)],
                    in_=P[:].rearrange("p (l two) -> p l two", two=2),
                    axis=mybir.AxisListType.X,
                    op=mybir.AluOpType.max,
                )

            Yb = Y[:, b * L_OUT : (b + 1) * L_OUT]
            if "prelu" in skip:
                nc.scalar.activation(
                    Yb,
                    Z[:],
                    mybir.ActivationFunctionType.Identity,
                    accum_out=sums[:, b : b + 1],
                )
            else:
                # PReLU with fused per-partition sum accumulation
                nc.scalar.activation(
                    Yb,
                    Z[:],
                    mybir.ActivationFunctionType.Prelu,
                    alpha=alpha_val,
                    accum_out=sums[:, b : b + 1],
                )
            if "ttr" in skip:
                nc.vector.memset(sumsqs[:, b : b + 1], float(L_OUT))
            else:
                # sum of squares: (y * 1.0) * y with fused per-partition sum
                SQ = sqp.tile([128, L_OUT], BF16)
                nc.vector.scalar_tensor_tensor(
                    out=SQ[:],
                    in0=Yb,
                    scalar=1.0,
                    in1=Yb,
                    op0=mybir.AluOpType.mult,
                    op1=mybir.AluOpType.mult,
                    accum_out=sumsqs[:, b : b + 1],
                )

        # ---------------- Phase 2: global BN stats + scale/shift ------------
        if n_batches < B_LOC:
            nc.vector.memset(sums[:, n_batches:B_LOC], 0.0)
            nc.vector.memset(sumsqs[:, n_batches:B_LOC], 0.0)
        sm = statsp.tile([128, 2], F32)
        nc.vector.tensor_reduce(
            sm[:, 0:1], sums[:], axis=mybir.AxisListType.X, op=mybir.AluOpType.add
        )
        nc.vector.tensor_reduce(
            sm[:, 1:2], sumsqs[:], axis=mybir.AxisListType.X, op=mybir.AluOpType.add
        )
        sg = statsp.tile([128, 2], F32)
        if "cc" in skip:
            nc.vector.tensor_scalar_mul(sg[:], sm[:], float(N_CORES))
        else:
            nc.sync.dma_start(out=cc_in[:], in_=sm[:])
            nc.gpsimd.collective_compute(
                "AllReduce",
                mybir.AluOpType.add,
                replica_groups=[list(range(N_CORES))],
                ins=[cc_in[:]],
                outs=[cc_out[:]],
            )
            nc.sync.dma_start(out=sg[:], in_=cc_out[:])

        inv_n = 1.0 / float(N_CORES * n_batches * L_OUT)
        mean = statsp.tile([128, 1], F32)
        nc.vector.tensor_scalar_mul(mean[:], sg[:, 0:1], inv_n)
        ve = statsp.tile([128, 1], F32)
        # ve = E[y^2] - mean^2 + eps   via (sg1*inv_n + eps) - mean^2
        e2 = statsp.tile([128, 1], F32)
        nc.vector.tensor_scalar(
            e2[:], sg[:, 1:2], inv_n, BN_EPS, mybir.AluOpType.mult, mybir.AluOpType.add
        )
        msq = statsp.tile([128, 1], F32)
        nc.vector.tensor_mul(msq[:], mean[:], mean[:])
        nc.vector.tensor_sub(ve[:], e2[:], msq[:])
        # rstd = 1/sqrt(ve), with one Newton step to clean up ACT sqrt error
        sq = statsp.tile([128, 1], F32)
        nc.scalar.activation(sq[:], ve[:], mybir.ActivationFunctionType.Sqrt)
        r0 = statsp.tile([128, 1], F32)
        nc.vector.reciprocal(r0[:], sq[:])
        rr = statsp.tile([128, 1], F32)
        nc.vector.tensor_mul(rr[:], r0[:], r0[:])
        nc.vector.tensor_mul(rr[:], rr[:], ve[:])
        nc.vector.tensor_scalar(
            rr[:], rr[:], -0.5, 1.5, mybir.AluOpType.mult, mybir.AluOpType.add
        )
        rstd = statsp.tile([128, 1], F32)
        nc.vector.tensor_mul(rstd[:], r0[:], rr[:])
        s_vec = statsp.tile([128, 1], F32)
        nc.vector.tensor_mul(s_vec[:], rstd[:], gamma_sb[:])
        t_vec = statsp.tile([128, 1], F32)
        nc.vector.tensor_mul(t_vec[:], mean[:], s_vec[:])
        nc.vector.tensor_sub(t_vec[:], beta_sb[:], t_vec[:])

        # ---------------- Phase 3: apply BN affine and store -----------------
        for b in range(n_batches):
            O = outp.tile([128, L_OUT], F32)
            nc.scalar.activation(
                O[:],
                Y[:, b * L_OUT : (b + 1) * L_OUT],
                mybir.ActivationFunctionType.Identity,
                bias=t_vec[:],
                scale=s_vec[:],
            )
            nc.sync.dma_start(out=out_ap[b], in_=O[:])

    nc.compile()
    return nc


def _prep_weights(W: np.ndarray) -> np.ndarray:
    sW = np.sign(W).astype(np.float32)  # [128, 64, 7]
    w_host = np.zeros((128, 4 * 128), dtype=np.float32)
    for t in range(3):
        w_host[0:64, 128 * t : 128 * (t + 1)] = sW[:, :, 2 * t].T
        w_host[64:128, 128 * t : 128 * (t + 1)] = sW[:, :, 2 * t + 1].T
    w_host[0:64, 384:512] = sW[:, :, 6].T
    return w_host.astype(ml_dtypes.float8_e4m3)


def kernel(x, W, alpha, gamma, beta):
    x = np.asarray(x, dtype=np.float32)
    W = np.asarray(W, dtype=np.float32)
    alpha_val = float(np.asarray(alpha).reshape(-1)[0])
    gamma = np.asarray(gamma, dtype=np.float32).reshape(128, 1)
    beta = np.asarray(beta, dtype=np.float32).reshape(128, 1)

    nc = _build_program(alpha_val)
    w_host = _prep_weights(W)

    in_maps = []
    for c in range(N_CORES):
        xs = np.ascontiguousarray(x[c * B_LOC : (c + 1) * B_LOC]).reshape(
            B_LOC, 128, L_IN // 2
        )
        in_maps.append({"x": xs, "w": w_host, "gamma": gamma, "beta": beta})

    res = run_bass_kernel_spmd(nc, in_maps, list(range(N_CORES)))
    out = np.concatenate([res.results[c]["out"] for c in range(N_CORES)], axis=0)
    return out.astype(np.float32)


if __name__ == "__main__":
    rng = np.random.default_rng(0)
    x = rng.standard_normal((B_FULL, C_IN, L_IN), dtype=np.float32)
    W = rng.standard_normal((C_OUT, C_IN, KSIZE), dtype=np.float32)
    alpha = np.full((1,), 0.25, np.float32)
    gamma = np.ones((C_OUT,), np.float32)
    beta = np.zeros((C_OUT,), np.float32)
    out = kernel(x=x, W=W, alpha=alpha, gamma=gamma, beta=beta)
    print(out.shape, out.dtype, float(out.mean()), float(out.std()))
